# revision 51
# baseline (speedup 1.0000x reference)
"""BiDAF attention-flow kernel for 8 Trainium2 NeuronCores (Bass/Tile).

Data-parallel over batch: B=32 -> 4 batches per core on 8 cores.

Math (per batch b):
  sim[i,j] = sp[i] + tp[j] + sum_d S[i,d]*wm[d]*T[j,d]
  A        = softmax_j(sim)
  source_target = A @ T
  w[i]     = exp(max_j sim[i,j]) ; tgt_attn = w / sum(w)
  target_source = tgt_attn @ S         (one row, broadcast over rows)
  out      = [S | source_target | S*source_target | S*target_source]

Final device strategy (device does ONLY the two big matmul passes):
  - sp/tp folded into the sim contraction as two extra host-built aff rows
    ([ones|tp], [sp|ones]); E^T = exp(sim^T) is one accumulating matmul chain
    + one EXP per j-chunk.  sp scales rows of E^T: it cancels in the A@T
    rowsum ratio and is wanted in the max path -> zero projection work.
  - aff lives in zero-padded [128,1024] SBUF singles (memset once, 18 rows
    DMA'd per batch via gpsimd/SWDGE so the tiny descriptors beat the big
    mega transfers into the DMA queues) so every matmul runs with the same
    128-row PE tile config (an 18-row weight load forces ~100ns of array
    reconfig twice).
  - E^T row max: 3 DVE max folds -> M ships to host, which finishes the tiny
    max/softmax/ts=attn@S chain in f32.
  - A@[T|1] accumulates st*rowsum | rowsum into per-ic PSUM banks, copied to
    SBUF bf16 (ACT/DVE alternating) and shipped UNNORMALIZED (host divides).
    The last batch splits each copy ACT||DVE and ships pair+single+single so
    the drain pipelines with the final matmuls.
  - PE stream: 4 junk warmup matmuls start the DVFS ramp while the first
    input DMA is in flight (512-col matmuls measure 427ns at the 1.2GHz
    p-state vs 216ns at 2.4GHz; the ramp only advances while PE stays
    continuously busy), then sim(b) / epi(b-1) interleave gap-free.
  - mega input is chunk-interleaved [stt_k|ttt_k]x3 | trow, shipped as 4
    per-chunk DMAs per batch so each sim kc-wave starts on partial arrival;
    all input triggers are emitted up front (a mega trigger queued on the
    sync sequencer behind a po output DMA would stall a later sim), and
    each dma_start costs ~0.6us on the shared HWDGE descriptor generator.
"""

import sys

import numpy as np
import ml_dtypes

try:
    import concourse.bass as bass
except ImportError:  # pragma: no cover
    sys.path.insert(0, "/opt/trn_rl_repo")
    import concourse.bass as bass

import concourse.mybir as mybir
import concourse.tile as tile
from concourse.bass_utils import run_bass_kernel_spmd

B, LS, LT, D = 32, 512, 512, 400
N_CORES = 8
BL = B // N_CORES  # batches per core
F32 = mybir.dt.float32
BF16 = mybir.dt.bfloat16
EXP = mybir.ActivationFunctionType.Exp

# mega: [stt_k | ttt_k] x3 | trow(4x401); aff ships separately (18 rows)
OTR = 3072
MEGA_W = 4676
N_WARM = 4


def _split_multi_waits(nc: bass.Bass) -> None:
    """This walrus build encodes at most ONE sync-wait per instruction.
    Tile's wait pass can attach several sem-waits to one instruction; hoist
    the extras onto same-engine NoOp carriers immediately before it."""
    ctr = 0
    for fn in nc.m.functions:
        for bb in fn.blocks:
            if not any(
                i.sync_info is not None and len(i.sync_info.on_wait) > 1
                for i in bb.instructions
            ):
                continue
            new_insts = []
            for inst in bb.instructions:
                si = inst.sync_info
                if si is not None and len(si.on_wait) > 1:
                    waits = list(si.on_wait)
                    for w in waits[:-1]:
                        ctr += 1
                        nop = mybir.InstNoOp(
                            name=f"splitw-{ctr}",
                            engine=inst.engine,
                            sync_info=mybir.SyncInfo(on_wait=[w], on_update=[]),
                            bass_nofuse=True,
                        )
                        nc.register_instruction(nop, overwrite=True)
                        new_insts.append(nop)
                    del si.on_wait[:-1]
                new_insts.append(inst)
            bb.instructions[:] = new_insts


def build_program() -> bass.Bass:
    nc = bass.Bass("TRN2", target_bir_lowering=False, debug=False)

    mega_h = nc.dram_tensor("mega", [BL, 128, MEGA_W], BF16, kind="ExternalInput").ap()
    aff_h = nc.dram_tensor("aff", [BL, 18, 1024], BF16, kind="ExternalInput").ap()
    po_h = nc.dram_tensor("po", [BL, 128, 4, 401], BF16, kind="ExternalOutput").ap()
    m_h = nc.dram_tensor("m", [BL, 128, 512], BF16, kind="ExternalOutput").ap()

    with tile.TileContext(nc) as tc:
        with (
            tc.tile_pool(name="singles", bufs=1) as singles,
            tc.tile_pool(name="pmega", bufs=4) as pmega,
            tc.tile_pool(name="pet", bufs=2) as pet,
            tc.tile_pool(name="pM", bufs=2) as pM,
            tc.tile_pool(name="pposb", bufs=2) as pposb,
            tc.tile_pool(name="psim_ps", bufs=4, space="PSUM") as psim_ps,
            tc.tile_pool(name="pepi_ps", bufs=4, space="PSUM") as pepi_ps,
        ):
            # junk memset first: the PE warmups wait on it
            junk = singles.tile([128, 512], BF16, tag="junk")
            nc.vector.memset(junk[:], 0.0)
            # zero-padded aff buffers (rows 18:128 stay zero forever); the
            # aff DMAs go through gpsimd/SWDGE so their descriptors beat the
            # big mega transfers into the queues
            affbuf = []
            for i in range(BL):
                ab = singles.tile([128, 1024], BF16, tag=f"aff{i}")
                nc.vector.memset(ab[:], 0.0)
                affbuf.append(ab)
            for i in range(BL):
                nc.gpsimd.dma_start(out=affbuf[i][0:18, :], in_=aff_h[i])

            # PE warmup: start the DVFS ramp while the first input DMA flies
            warm = psim_ps.tile([128, 512], F32, tag="psim")
            for _ in range(N_WARM):
                nc.tensor.matmul(
                    warm[:], lhsT=junk[:, 0:128], rhs=junk[:], start=True, stop=True
                )
            wsink = singles.tile([128, 8], F32, tag="wsink")
            nc.vector.tensor_copy(wsink[:], warm[:, 0:8])

            state = {}

            def chunks_dma(b):
                """Contraction-side inputs for batch b (sim needs these)."""
                mega = pmega.tile([128, MEGA_W], BF16, tag="mega")
                # per-chunk transfers: each sim kc-wave starts on partial
                # arrival instead of waiting the whole batch transfer
                nc.sync.dma_start(out=mega[:, 0:1024], in_=mega_h[b][:, 0:1024])
                nc.sync.dma_start(out=mega[:, 1024:2048], in_=mega_h[b][:, 1024:2048])
                nc.sync.dma_start(out=mega[:, 2048:OTR], in_=mega_h[b][:, 2048:OTR])
                state[b] = dict(mega=mega)

            def trow_dma(b):
                """Row-side input for batch b (epi needs it ~6us later)."""
                mega = state[b]["mega"]
                nc.sync.dma_start(out=mega[:, OTR:MEGA_W], in_=mega_h[b][:, OTR:MEGA_W])

            def sim_pass(b):
                """E^T = exp(sim^T) (sp/tp folded into the aff rows) + max
                folds; M ships to host."""
                st = state[b]
                mega = st["mega"]
                et = pet.tile([128, 4, 512], BF16, tag="et")
                st["et"] = et

                def sim_mm(ps, jc, kc):
                    nc.tensor.matmul(
                        ps[:],
                        lhsT=mega[
                            :, 1024 * kc + 512 + jc * 128 : 1024 * kc + 512 + (jc + 1) * 128
                        ],
                        rhs=mega[:, 1024 * kc : 1024 * kc + 512],
                        start=(kc == 0),
                        stop=False,
                    )

                def aff_mm(ps, jc):
                    nc.tensor.matmul(
                        ps[:],
                        lhsT=affbuf[b][:, 512 + jc * 128 : 512 + (jc + 1) * 128],
                        rhs=affbuf[b][:, 0:512],
                        start=False,
                        stop=True,
                    )

                for jc in range(4):
                    ps = psim_ps.tile([128, 512], F32, tag="psim")
                    for kc in range(3):
                        sim_mm(ps, jc, kc)
                    aff_mm(ps, jc)
                    nc.scalar.activation(et[:, jc, :], ps[:], EXP)
                M = pM.tile([128, 512], BF16, tag="M")
                nc.vector.tensor_max(M[:], et[:, 0, :], et[:, 1, :])
                nc.vector.tensor_max(M[:], M[:], et[:, 2, :])
                nc.vector.tensor_max(M[:], M[:], et[:, 3, :])
                nc.gpsimd.dma_start(out=m_h[b], in_=M[:])

            def epi_pass(b):
                """po = (E^T)^T @ [T|1] (st*rowsum | rowsum), copied to SBUF
                bf16 (alternating ACT/DVE) and shipped raw; host normalizes."""
                st = state[b]
                mega, et = st["mega"], st["et"]
                posb = pposb.tile([128, 4, 401], BF16, tag="posb")
                for ic in range(4):
                    po = pepi_ps.tile([128, 401], F32, tag="pepi")
                    for jc in range(4):
                        nc.tensor.matmul(
                            po[:],
                            lhsT=et[:, jc, ic * 128 : (ic + 1) * 128],
                            rhs=mega[:, OTR + jc * 401 : OTR + (jc + 1) * 401],
                            start=(jc == 0),
                            stop=(jc == 3),
                        )
                    if b == BL - 1:
                        # split each copy ACT/DVE so the drain pipelines
                        nc.scalar.copy(posb[:, ic, 0:200], po[:, 0:200])
                        nc.vector.tensor_copy(posb[:, ic, 200:401], po[:, 200:401])
                        if ic == 1:
                            nc.sync.dma_start(
                                out=po_h[b][:, 0:2], in_=posb[:, 0:2, :]
                            )
                        elif ic >= 2:
                            nc.sync.dma_start(out=po_h[b][:, ic], in_=posb[:, ic, :])
                    elif ic % 2 == 0:
                        nc.scalar.copy(posb[:, ic, :], po[:])
                    else:
                        nc.vector.tensor_copy(posb[:, ic, :], po[:])
                if b < BL - 1:
                    nc.sync.dma_start(out=po_h[b], in_=posb[:])

            # all input DMAs triggered up front (a mega trigger queued on the
            # sync sequencer behind a po output DMA would stall sim(b)), with
            # each trow deferred one batch so early chunk transfers land first
            for b in range(BL):
                chunks_dma(b)
                if b >= 1:
                    trow_dma(b - 1)
            trow_dma(BL - 1)
            sim_pass(0)
            for b in range(1, BL):
                sim_pass(b)
                epi_pass(b - 1)
            epi_pass(BL - 1)
    return nc


_NC_CACHE: list = []


def _get_program() -> bass.Bass:
    if not _NC_CACHE:
        nc = build_program()
        _split_multi_waits(nc)
        _NC_CACHE.append(nc)
    return _NC_CACHE[0]


def _host_shards(S: np.ndarray, T: np.ndarray, w: np.ndarray):
    """Build per-core input maps (layout marshalling + tiny projections)."""
    bf16 = ml_dtypes.bfloat16
    ws, wt, wm = w[:D], w[D : 2 * D], w[2 * D :]
    sp = S @ ws  # [B, 512]
    tp = T @ wt  # [B, 512]
    # row blocks: i = 4p + ic
    A = S.reshape(B, 128, 4, D)
    Bt = T.reshape(B, 128, 4, D)
    # transposed cols: c = ic*128 + p  ->  i = 4p + ic ; d rows 3p+k for d<384
    StP = A.transpose(0, 3, 2, 1).reshape(B, D, 512)
    TtP = Bt.transpose(0, 3, 2, 1).reshape(B, D, 512) * wm[None, :, None]
    stt = StP[:, 0:384].reshape(B, 128, 3, 512)
    ttt = TtP[:, 0:384].reshape(B, 128, 3, 512)
    trow = np.empty((B, 128, 4, 401), np.float32)
    trow[:, :, :, 0:400] = Bt
    trow[:, :, :, 400] = 1.0
    mega = np.empty((B, 128, MEGA_W), np.float32)
    for k in range(3):
        mega[:, :, 1024 * k : 1024 * k + 512] = stt[:, :, k]
        mega[:, :, 1024 * k + 512 : 1024 * (k + 1)] = ttt[:, :, k]
    mega[:, :, OTR:MEGA_W] = trow.reshape(B, 128, 1604)
    mega = mega.astype(bf16)

    # aff rows; projections in c-order: c = ic*128+p <-> i = 4p+ic
    sp_c = sp.reshape(B, 128, 4).transpose(0, 2, 1).reshape(B, 512)
    tp_c = tp.reshape(B, 128, 4).transpose(0, 2, 1).reshape(B, 512)
    aff = np.empty((B, 18, 1024), np.float32)
    aff[:, 0:16, 0:512] = StP[:, 384:400]
    aff[:, 0:16, 512:1024] = TtP[:, 384:400]
    aff[:, 16, 0:512] = 1.0
    aff[:, 16, 512:1024] = tp_c
    aff[:, 17, 0:512] = sp_c
    aff[:, 17, 512:1024] = 1.0
    aff = aff.astype(bf16)

    in_maps = []
    for c in range(N_CORES):
        sl = slice(c * BL, (c + 1) * BL)
        in_maps.append({"mega": mega[sl], "aff": aff[sl]})
    return in_maps


def kernel(source_embedding, target_embedding, w_sim, **run_kwargs):
    S = np.asarray(source_embedding, dtype=np.float32)
    T = np.asarray(target_embedding, dtype=np.float32)
    w = np.asarray(w_sim, dtype=np.float32)
    assert S.shape == (B, LS, D) and T.shape == (B, LT, D) and w.shape == (3 * D,)

    nc = _get_program()
    in_maps = _host_shards(S, T, w)
    res = run_bass_kernel_spmd(nc, in_maps, core_ids=list(range(N_CORES)), **run_kwargs)

    out = np.empty((B, LS, 1600), np.float32)
    out[:, :, 0:400] = S
    for c in range(N_CORES):
        sl = slice(c * BL, (c + 1) * BL)
        po = (
            np.asarray(res.results[c]["po"])
            .astype(np.float32)
            .reshape(BL, 512, 401)
        )  # rows i = 4p+ic
        st = po[:, :, 0:400] / po[:, :, 400:401]
        u_c = np.asarray(res.results[c]["m"]).astype(np.float32).max(axis=1)
        u = u_c.reshape(BL, 4, 128).transpose(0, 2, 1).reshape(BL, 512)
        attn = u / u.sum(axis=1, keepdims=True)  # [BL, 512]
        ts = np.einsum("bi,bid->bd", attn, S[sl])  # [BL, 400]
        out[sl, :, 400:800] = st
        out[sl, :, 800:1200] = S[sl] * st
        out[sl, :, 1200:1600] = S[sl] * ts[:, None, :]
    if run_kwargs:
        kernel.last_results = res  # expose profile info to test harness
    return out


# revision 55
# speedup vs baseline: 1.1429x; 1.1429x over previous
"""BiDAF attention-flow kernel for 8 Trainium2 NeuronCores (Bass/Tile).

Data-parallel over batch: B=32 -> 4 batches per core on 8 cores.

Math (per batch b):
  sim[i,j] = sp[i] + tp[j] + sum_d S[i,d]*wm[d]*T[j,d]
  A        = softmax_j(sim)
  source_target = A @ T
  w[i]     = exp(max_j sim[i,j]) ; tgt_attn = w / sum(w)
  target_source = tgt_attn @ S         (one row, broadcast over rows)
  out      = [S | source_target | S*source_target | S*target_source]

Final device strategy (device does ONLY the two big matmul passes):
  - sp/tp folded into the sim contraction as two extra host-built aff rows
    ([ones|tp], [sp|ones]); E^T = exp(sim^T) is one accumulating matmul chain
    + one EXP per j-chunk.  sp scales rows of E^T: it cancels in the A@T
    rowsum ratio and is wanted in the max path -> zero projection work.
  - aff lives in zero-padded [128,1024] SBUF singles (memset once, 18 rows
    DMA'd per batch via gpsimd/SWDGE so the tiny descriptors beat the big
    mega transfers into the DMA queues) so every matmul runs with the same
    128-row PE tile config (an 18-row weight load forces ~100ns of array
    reconfig twice).
  - E^T row max: 3 DVE max folds -> M ships to host, which finishes the tiny
    max/softmax/ts=attn@S chain in f32.
  - A@[T|1] accumulates st*rowsum | rowsum into per-ic PSUM banks, copied to
    SBUF bf16 (ACT/DVE alternating) and shipped UNNORMALIZED (host divides).
    The last batch splits each copy ACT||DVE and ships pair+single+single so
    the drain pipelines with the final matmuls.
  - PE stream: 4 junk warmup matmuls start the DVFS ramp while the first
    input DMA is in flight (512-col matmuls measure 427ns at the 1.2GHz
    p-state vs 216ns at 2.4GHz; the ramp only advances while PE stays
    continuously busy), then sim(b) / epi(b-1) interleave gap-free.
  - mega input is chunk-interleaved [stt_k|ttt_k]x3 | trow, shipped as 4
    per-chunk DMAs per batch so each sim kc-wave starts on partial arrival;
    all input triggers are emitted up front (a mega trigger queued on the
    sync sequencer behind a po output DMA would stall a later sim), and
    each dma_start costs ~0.6us on the shared HWDGE descriptor generator.
"""

import sys

import numpy as np
import ml_dtypes

try:
    import concourse.bass as bass
except ImportError:  # pragma: no cover
    sys.path.insert(0, "/opt/trn_rl_repo")
    import concourse.bass as bass

import concourse.mybir as mybir
import concourse.tile as tile
from concourse.bass_utils import run_bass_kernel_spmd

B, LS, LT, D = 32, 512, 512, 400
N_CORES = 8
BL = B // N_CORES  # batches per core
F32 = mybir.dt.float32
BF16 = mybir.dt.bfloat16
EXP = mybir.ActivationFunctionType.Exp

# mega: [stt_k | ttt_k] x3 | trow(4x401); aff ships separately (18 rows)
OTR = 3072
MEGA_W = 4676
N_WARM = 4


def _split_multi_waits(nc: bass.Bass) -> None:
    """This walrus build encodes at most ONE sync-wait per instruction.
    Tile's wait pass can attach several sem-waits to one instruction; hoist
    the extras onto same-engine NoOp carriers immediately before it."""
    ctr = 0
    for fn in nc.m.functions:
        for bb in fn.blocks:
            if not any(
                i.sync_info is not None and len(i.sync_info.on_wait) > 1
                for i in bb.instructions
            ):
                continue
            new_insts = []
            for inst in bb.instructions:
                si = inst.sync_info
                if si is not None and len(si.on_wait) > 1:
                    waits = list(si.on_wait)
                    for w in waits[:-1]:
                        ctr += 1
                        nop = mybir.InstNoOp(
                            name=f"splitw-{ctr}",
                            engine=inst.engine,
                            sync_info=mybir.SyncInfo(on_wait=[w], on_update=[]),
                            bass_nofuse=True,
                        )
                        nc.register_instruction(nop, overwrite=True)
                        new_insts.append(nop)
                    del si.on_wait[:-1]
                new_insts.append(inst)
            bb.instructions[:] = new_insts


def build_program() -> bass.Bass:
    nc = bass.Bass("TRN2", target_bir_lowering=False, debug=False)

    mega_h = nc.dram_tensor("mega", [BL, 128, MEGA_W], BF16, kind="ExternalInput").ap()
    aff_h = nc.dram_tensor("aff", [BL, 18, 1024], BF16, kind="ExternalInput").ap()
    po_h = nc.dram_tensor("po", [BL, 128, 4, 401], BF16, kind="ExternalOutput").ap()
    m_h = nc.dram_tensor("m", [BL, 128, 512], BF16, kind="ExternalOutput").ap()

    with tile.TileContext(nc) as tc:
        with (
            tc.tile_pool(name="singles", bufs=1) as singles,
            tc.tile_pool(name="pmega", bufs=4) as pmega,
            tc.tile_pool(name="pet", bufs=2) as pet,
            tc.tile_pool(name="pM", bufs=2) as pM,
            tc.tile_pool(name="pposb", bufs=2) as pposb,
            tc.tile_pool(name="psim_ps", bufs=4, space="PSUM") as psim_ps,
            tc.tile_pool(name="pepi_ps", bufs=4, space="PSUM") as pepi_ps,
        ):
            state = {}
            # batch 0's first chunk rides gpsimd/SWDGE as the Pool engine's
            # FIRST instruction: Pool starts earliest, so the DMA pipeline
            # spins up sooner and sim(0) starts earlier
            mega0 = pmega.tile([128, MEGA_W], BF16, tag="mega")
            nc.gpsimd.dma_start(out=mega0[:, 0:1024], in_=mega_h[0][:, 0:1024])
            state[0] = dict(mega=mega0)

            # junk memset first: the PE warmups wait on it
            junk = singles.tile([128, 512], BF16, tag="junk")
            nc.vector.memset(junk[:], 0.0)
            # zero-padded aff buffers (rows 18:128 stay zero forever); the
            # aff DMAs go through gpsimd/SWDGE so their descriptors beat the
            # big mega transfers into the queues
            affbuf = []
            for i in range(BL):
                ab = singles.tile([128, 1024], BF16, tag=f"aff{i}")
                nc.vector.memset(ab[:], 0.0)
                affbuf.append(ab)
            for i in range(BL):
                nc.gpsimd.dma_start(out=affbuf[i][0:18, :], in_=aff_h[i])

            # PE warmup: start the DVFS ramp while the first input DMA flies
            warm = psim_ps.tile([128, 512], F32, tag="psim")
            for _ in range(N_WARM):
                nc.tensor.matmul(
                    warm[:], lhsT=junk[:, 0:128], rhs=junk[:], start=True, stop=True
                )
            wsink = singles.tile([128, 8], F32, tag="wsink")
            nc.vector.tensor_copy(wsink[:], warm[:, 0:8])

            def chunks_dma(b):
                """Contraction-side inputs for batch b (sim needs these)."""
                if b == 0:
                    mega = state[0]["mega"]  # chunk0 already in flight (Pool)
                else:
                    mega = pmega.tile([128, MEGA_W], BF16, tag="mega")
                    nc.sync.dma_start(out=mega[:, 0:1024], in_=mega_h[b][:, 0:1024])
                nc.sync.dma_start(out=mega[:, 1024:2048], in_=mega_h[b][:, 1024:2048])
                nc.sync.dma_start(out=mega[:, 2048:OTR], in_=mega_h[b][:, 2048:OTR])
                state[b] = dict(mega=mega)

            def trow_dma(b):
                """Row-side input for batch b (epi needs it ~6us later)."""
                mega = state[b]["mega"]
                nc.sync.dma_start(out=mega[:, OTR:MEGA_W], in_=mega_h[b][:, OTR:MEGA_W])

            def sim_pass(b):
                """E^T = exp(sim^T) (sp/tp folded into the aff rows) + max
                folds; M ships to host."""
                st = state[b]
                mega = st["mega"]
                et = pet.tile([128, 4, 512], BF16, tag="et")
                st["et"] = et

                def sim_mm(ps, jc, kc):
                    nc.tensor.matmul(
                        ps[:],
                        lhsT=mega[
                            :, 1024 * kc + 512 + jc * 128 : 1024 * kc + 512 + (jc + 1) * 128
                        ],
                        rhs=mega[:, 1024 * kc : 1024 * kc + 512],
                        start=(kc == 0),
                        stop=False,
                    )

                def aff_mm(ps, jc):
                    nc.tensor.matmul(
                        ps[:],
                        lhsT=affbuf[b][:, 512 + jc * 128 : 512 + (jc + 1) * 128],
                        rhs=affbuf[b][:, 0:512],
                        start=False,
                        stop=True,
                    )

                for jc in range(4):
                    ps = psim_ps.tile([128, 512], F32, tag="psim")
                    for kc in range(3):
                        sim_mm(ps, jc, kc)
                    aff_mm(ps, jc)
                    nc.scalar.activation(et[:, jc, :], ps[:], EXP)
                M = pM.tile([128, 512], BF16, tag="M")
                nc.vector.tensor_max(M[:], et[:, 0, :], et[:, 1, :])
                nc.vector.tensor_max(M[:], M[:], et[:, 2, :])
                nc.vector.tensor_max(M[:], M[:], et[:, 3, :])
                nc.gpsimd.dma_start(out=m_h[b], in_=M[:])

            def epi_pass(b):
                """po = (E^T)^T @ [T|1] (st*rowsum | rowsum), copied to SBUF
                bf16 (alternating ACT/DVE) and shipped raw; host normalizes."""
                st = state[b]
                mega, et = st["mega"], st["et"]
                posb = pposb.tile([128, 4, 401], BF16, tag="posb")
                for ic in range(4):
                    po = pepi_ps.tile([128, 401], F32, tag="pepi")
                    for jc in range(4):
                        nc.tensor.matmul(
                            po[:],
                            lhsT=et[:, jc, ic * 128 : (ic + 1) * 128],
                            rhs=mega[:, OTR + jc * 401 : OTR + (jc + 1) * 401],
                            start=(jc == 0),
                            stop=(jc == 3),
                        )
                    if b == BL - 1:
                        # split each copy ACT/DVE so the drain pipelines
                        nc.scalar.copy(posb[:, ic, 0:200], po[:, 0:200])
                        nc.vector.tensor_copy(posb[:, ic, 200:401], po[:, 200:401])
                        if ic == 1:
                            nc.sync.dma_start(
                                out=po_h[b][:, 0:2], in_=posb[:, 0:2, :]
                            )
                        elif ic >= 2:
                            nc.sync.dma_start(out=po_h[b][:, ic], in_=posb[:, ic, :])
                    elif ic % 2 == 0:
                        nc.scalar.copy(posb[:, ic, :], po[:])
                    else:
                        nc.vector.tensor_copy(posb[:, ic, :], po[:])
                if b < BL - 1:
                    nc.sync.dma_start(out=po_h[b], in_=posb[:])

            # all input DMAs triggered up front (a mega trigger queued on the
            # sync sequencer behind a po output DMA would stall sim(b)), with
            # each trow deferred one batch so early chunk transfers land first
            for b in range(BL):
                chunks_dma(b)
                if b >= 1:
                    trow_dma(b - 1)
            trow_dma(BL - 1)
            sim_pass(0)
            for b in range(1, BL):
                sim_pass(b)
                epi_pass(b - 1)
            epi_pass(BL - 1)
    return nc


_NC_CACHE: list = []


def _get_program() -> bass.Bass:
    if not _NC_CACHE:
        nc = build_program()
        _split_multi_waits(nc)
        _NC_CACHE.append(nc)
    return _NC_CACHE[0]


def _host_shards(S: np.ndarray, T: np.ndarray, w: np.ndarray):
    """Build per-core input maps (layout marshalling + tiny projections)."""
    bf16 = ml_dtypes.bfloat16
    ws, wt, wm = w[:D], w[D : 2 * D], w[2 * D :]
    sp = S @ ws  # [B, 512]
    tp = T @ wt  # [B, 512]
    # row blocks: i = 4p + ic
    A = S.reshape(B, 128, 4, D)
    Bt = T.reshape(B, 128, 4, D)
    # transposed cols: c = ic*128 + p  ->  i = 4p + ic ; d rows 3p+k for d<384
    StP = A.transpose(0, 3, 2, 1).reshape(B, D, 512)
    TtP = Bt.transpose(0, 3, 2, 1).reshape(B, D, 512) * wm[None, :, None]
    stt = StP[:, 0:384].reshape(B, 128, 3, 512)
    ttt = TtP[:, 0:384].reshape(B, 128, 3, 512)
    trow = np.empty((B, 128, 4, 401), np.float32)
    trow[:, :, :, 0:400] = Bt
    trow[:, :, :, 400] = 1.0
    mega = np.empty((B, 128, MEGA_W), np.float32)
    for k in range(3):
        mega[:, :, 1024 * k : 1024 * k + 512] = stt[:, :, k]
        mega[:, :, 1024 * k + 512 : 1024 * (k + 1)] = ttt[:, :, k]
    mega[:, :, OTR:MEGA_W] = trow.reshape(B, 128, 1604)
    mega = mega.astype(bf16)

    # aff rows; projections in c-order: c = ic*128+p <-> i = 4p+ic
    sp_c = sp.reshape(B, 128, 4).transpose(0, 2, 1).reshape(B, 512)
    tp_c = tp.reshape(B, 128, 4).transpose(0, 2, 1).reshape(B, 512)
    aff = np.empty((B, 18, 1024), np.float32)
    aff[:, 0:16, 0:512] = StP[:, 384:400]
    aff[:, 0:16, 512:1024] = TtP[:, 384:400]
    aff[:, 16, 0:512] = 1.0
    aff[:, 16, 512:1024] = tp_c
    aff[:, 17, 0:512] = sp_c
    aff[:, 17, 512:1024] = 1.0
    aff = aff.astype(bf16)

    in_maps = []
    for c in range(N_CORES):
        sl = slice(c * BL, (c + 1) * BL)
        in_maps.append({"mega": mega[sl], "aff": aff[sl]})
    return in_maps


def kernel(source_embedding, target_embedding, w_sim, **run_kwargs):
    S = np.asarray(source_embedding, dtype=np.float32)
    T = np.asarray(target_embedding, dtype=np.float32)
    w = np.asarray(w_sim, dtype=np.float32)
    assert S.shape == (B, LS, D) and T.shape == (B, LT, D) and w.shape == (3 * D,)

    nc = _get_program()
    in_maps = _host_shards(S, T, w)
    res = run_bass_kernel_spmd(nc, in_maps, core_ids=list(range(N_CORES)), **run_kwargs)

    out = np.empty((B, LS, 1600), np.float32)
    out[:, :, 0:400] = S
    for c in range(N_CORES):
        sl = slice(c * BL, (c + 1) * BL)
        po = (
            np.asarray(res.results[c]["po"])
            .astype(np.float32)
            .reshape(BL, 512, 401)
        )  # rows i = 4p+ic
        st = po[:, :, 0:400] / po[:, :, 400:401]
        u_c = np.asarray(res.results[c]["m"]).astype(np.float32).max(axis=1)
        u = u_c.reshape(BL, 4, 128).transpose(0, 2, 1).reshape(BL, 512)
        attn = u / u.sum(axis=1, keepdims=True)  # [BL, 512]
        ts = np.einsum("bi,bid->bd", attn, S[sl])  # [BL, 400]
        out[sl, :, 400:800] = st
        out[sl, :, 800:1200] = S[sl] * st
        out[sl, :, 1200:1600] = S[sl] * ts[:, None, :]
    if run_kwargs:
        kernel.last_results = res  # expose profile info to test harness
    return out


# revision 57
# speedup vs baseline: 1.1844x; 1.0363x over previous
"""BiDAF attention-flow kernel for 8 Trainium2 NeuronCores (Bass/Tile).

Data-parallel over batch: B=32 -> 4 batches per core on 8 cores.

Math (per batch b):
  sim[i,j] = sp[i] + tp[j] + sum_d S[i,d]*wm[d]*T[j,d]
  A        = softmax_j(sim)
  source_target = A @ T
  w[i]     = exp(max_j sim[i,j]) ; tgt_attn = w / sum(w)
  target_source = tgt_attn @ S         (one row, broadcast over rows)
  out      = [S | source_target | S*source_target | S*target_source]

Final device strategy (device does ONLY the two big matmul passes):
  - sp/tp folded into the sim contraction as two extra host-built aff rows
    ([ones|tp], [sp|ones]); E^T = exp(sim^T) is one accumulating matmul chain
    + one EXP per j-chunk.  sp scales rows of E^T: it cancels in the A@T
    rowsum ratio and is wanted in the max path -> zero projection work.
  - aff lives in zero-padded [128,1024] SBUF singles (memset once, 18 rows
    DMA'd per batch via gpsimd/SWDGE so the tiny descriptors beat the big
    mega transfers into the DMA queues) so every matmul runs with the same
    128-row PE tile config (an 18-row weight load forces ~100ns of array
    reconfig twice).
  - E^T row max: 3 DVE max folds -> M ships to host, which finishes the tiny
    max/softmax/ts=attn@S chain in f32.
  - A@[T|1] accumulates st*rowsum | rowsum into per-ic PSUM banks, copied to
    SBUF bf16 (ACT/DVE alternating) and shipped UNNORMALIZED (host divides).
    The last batch splits each copy ACT||DVE and ships pair+single+single so
    the drain pipelines with the final matmuls.
  - PE stream: 4 junk warmup matmuls start the DVFS ramp while the first
    input DMA is in flight (512-col matmuls measure 427ns at the 1.2GHz
    p-state vs 216ns at 2.4GHz; the ramp only advances while PE stays
    continuously busy), then sim(b) / epi(b-1) interleave gap-free.
  - mega input is chunk-interleaved [stt_k|ttt_k]x3 | trow, shipped as 4
    per-chunk DMAs per batch so each sim kc-wave starts on partial arrival;
    all input triggers are emitted up front (a mega trigger queued on the
    sync sequencer behind a po output DMA would stall a later sim), and
    each dma_start costs ~0.6us on the shared HWDGE descriptor generator.
"""

import sys

import numpy as np
import ml_dtypes

try:
    import concourse.bass as bass
except ImportError:  # pragma: no cover
    sys.path.insert(0, "/opt/trn_rl_repo")
    import concourse.bass as bass

import concourse.mybir as mybir
import concourse.tile as tile
from concourse.bass_utils import run_bass_kernel_spmd

B, LS, LT, D = 32, 512, 512, 400
N_CORES = 8
BL = B // N_CORES  # batches per core
F32 = mybir.dt.float32
BF16 = mybir.dt.bfloat16
EXP = mybir.ActivationFunctionType.Exp

# mega: [stt_k | ttt_k] x3 | trow(4x401); aff ships separately (18 rows)
OTR = 3072
MEGA_W = 4676
N_WARM = 4


def _split_multi_waits(nc: bass.Bass) -> None:
    """This walrus build encodes at most ONE sync-wait per instruction.
    Tile's wait pass can attach several sem-waits to one instruction; hoist
    the extras onto same-engine NoOp carriers immediately before it."""
    ctr = 0
    for fn in nc.m.functions:
        for bb in fn.blocks:
            if not any(
                i.sync_info is not None and len(i.sync_info.on_wait) > 1
                for i in bb.instructions
            ):
                continue
            new_insts = []
            for inst in bb.instructions:
                si = inst.sync_info
                if si is not None and len(si.on_wait) > 1:
                    waits = list(si.on_wait)
                    for w in waits[:-1]:
                        ctr += 1
                        nop = mybir.InstNoOp(
                            name=f"splitw-{ctr}",
                            engine=inst.engine,
                            sync_info=mybir.SyncInfo(on_wait=[w], on_update=[]),
                            bass_nofuse=True,
                        )
                        nc.register_instruction(nop, overwrite=True)
                        new_insts.append(nop)
                    del si.on_wait[:-1]
                new_insts.append(inst)
            bb.instructions[:] = new_insts


def build_program() -> bass.Bass:
    nc = bass.Bass("TRN2", target_bir_lowering=False, debug=False)

    mega_h = nc.dram_tensor("mega", [BL, 128, MEGA_W], BF16, kind="ExternalInput").ap()
    aff_h = nc.dram_tensor("aff", [BL, 18, 1024], BF16, kind="ExternalInput").ap()
    po_h = nc.dram_tensor("po", [BL, 128, 4, 401], BF16, kind="ExternalOutput").ap()
    m_h = nc.dram_tensor("m", [BL, 128, 512], BF16, kind="ExternalOutput").ap()

    with tile.TileContext(nc) as tc:
        with (
            tc.tile_pool(name="singles", bufs=1) as singles,
            tc.tile_pool(name="pmega", bufs=4) as pmega,
            tc.tile_pool(name="pet", bufs=2) as pet,
            tc.tile_pool(name="pM", bufs=2) as pM,
            tc.tile_pool(name="pposb", bufs=2) as pposb,
            tc.tile_pool(name="psim_ps", bufs=4, space="PSUM") as psim_ps,
            tc.tile_pool(name="pepi_ps", bufs=4, space="PSUM") as pepi_ps,
        ):
            state = {}
            # junk memset first: the PE warmups wait on it
            junk = singles.tile([128, 512], BF16, tag="junk")
            nc.vector.memset(junk[:], 0.0)
            # zero-padded aff buffers (rows 18:128 stay zero forever); the
            # aff DMAs go through gpsimd/SWDGE so their descriptors beat the
            # big mega transfers into the queues
            affbuf = []
            for i in range(BL):
                ab = singles.tile([128, 1024], BF16, tag=f"aff{i}")
                nc.vector.memset(ab[:], 0.0)
                affbuf.append(ab)
            for i in range(BL):
                nc.gpsimd.dma_start(out=affbuf[i][0:18, :], in_=aff_h[i])

            # PE warmup: start the DVFS ramp while the first input DMA flies
            warm = psim_ps.tile([128, 512], F32, tag="psim")
            for _ in range(N_WARM):
                nc.tensor.matmul(
                    warm[:], lhsT=junk[:, 0:128], rhs=junk[:], start=True, stop=True
                )
            wsink = singles.tile([128, 8], F32, tag="wsink")
            nc.vector.tensor_copy(wsink[:], warm[:, 0:8])

            def chunks_dma(b):
                """Contraction-side inputs for batch b (sim needs these)."""
                mega = pmega.tile([128, MEGA_W], BF16, tag="mega")
                # per-chunk transfers: each sim kc-wave starts on partial
                # arrival instead of waiting the whole batch transfer
                nc.sync.dma_start(out=mega[:, 0:1024], in_=mega_h[b][:, 0:1024])
                nc.sync.dma_start(out=mega[:, 1024:2048], in_=mega_h[b][:, 1024:2048])
                nc.sync.dma_start(out=mega[:, 2048:OTR], in_=mega_h[b][:, 2048:OTR])
                state[b] = dict(mega=mega)

            def trow_dma(b):
                """Row-side input for batch b (epi needs it ~6us later)."""
                mega = state[b]["mega"]
                nc.sync.dma_start(out=mega[:, OTR:MEGA_W], in_=mega_h[b][:, OTR:MEGA_W])

            def sim_pass(b):
                """E^T = exp(sim^T) (sp/tp folded into the aff rows) + max
                folds; M ships to host."""
                st = state[b]
                mega = st["mega"]
                et = pet.tile([128, 4, 512], BF16, tag="et")
                st["et"] = et

                def sim_mm(ps, jc, kc):
                    nc.tensor.matmul(
                        ps[:],
                        lhsT=mega[
                            :, 1024 * kc + 512 + jc * 128 : 1024 * kc + 512 + (jc + 1) * 128
                        ],
                        rhs=mega[:, 1024 * kc : 1024 * kc + 512],
                        start=(kc == 0),
                        stop=False,
                    )

                def aff_mm(ps, jc):
                    nc.tensor.matmul(
                        ps[:],
                        lhsT=affbuf[b][:, 512 + jc * 128 : 512 + (jc + 1) * 128],
                        rhs=affbuf[b][:, 0:512],
                        start=False,
                        stop=True,
                    )

                for jc in range(4):
                    ps = psim_ps.tile([128, 512], F32, tag="psim")
                    for kc in range(3):
                        sim_mm(ps, jc, kc)
                    aff_mm(ps, jc)
                    nc.scalar.activation(et[:, jc, :], ps[:], EXP)
                M = pM.tile([128, 512], BF16, tag="M")
                nc.vector.tensor_max(M[:], et[:, 0, :], et[:, 1, :])
                nc.vector.tensor_max(M[:], M[:], et[:, 2, :])
                nc.vector.tensor_max(M[:], M[:], et[:, 3, :])
                nc.gpsimd.dma_start(out=m_h[b], in_=M[:])

            def epi_pass(b):
                """po = (E^T)^T @ [T|1] (st*rowsum | rowsum), copied to SBUF
                bf16 (alternating ACT/DVE) and shipped raw; host normalizes."""
                st = state[b]
                mega, et = st["mega"], st["et"]
                posb = pposb.tile([128, 4, 401], BF16, tag="posb")
                for ic in range(4):
                    po = pepi_ps.tile([128, 401], F32, tag="pepi")
                    for jc in range(4):
                        nc.tensor.matmul(
                            po[:],
                            lhsT=et[:, jc, ic * 128 : (ic + 1) * 128],
                            rhs=mega[:, OTR + jc * 401 : OTR + (jc + 1) * 401],
                            start=(jc == 0),
                            stop=(jc == 3),
                        )
                    if b == BL - 1:
                        # split each copy ACT/DVE so the drain pipelines
                        nc.scalar.copy(posb[:, ic, 0:200], po[:, 0:200])
                        nc.vector.tensor_copy(posb[:, ic, 200:401], po[:, 200:401])
                        if ic == 1:
                            nc.sync.dma_start(
                                out=po_h[b][:, 0:2], in_=posb[:, 0:2, :]
                            )
                        elif ic >= 2:
                            nc.sync.dma_start(out=po_h[b][:, ic], in_=posb[:, ic, :])
                    elif ic % 2 == 0:
                        nc.scalar.copy(posb[:, ic, :], po[:])
                    else:
                        nc.vector.tensor_copy(posb[:, ic, :], po[:])
                if b < BL - 1:
                    nc.sync.dma_start(out=po_h[b], in_=posb[:])

            # all input DMAs triggered up front (a mega trigger queued on the
            # sync sequencer behind a po output DMA would stall sim(b)), with
            # each trow deferred one batch so early chunk transfers land first
            for b in range(BL):
                chunks_dma(b)
                if b >= 1:
                    trow_dma(b - 1)
            trow_dma(BL - 1)
            sim_pass(0)
            for b in range(1, BL):
                sim_pass(b)
                epi_pass(b - 1)
            epi_pass(BL - 1)
    return nc


_NC_CACHE: list = []


def _get_program() -> bass.Bass:
    if not _NC_CACHE:
        nc = build_program()
        _split_multi_waits(nc)
        _NC_CACHE.append(nc)
    return _NC_CACHE[0]


def _host_shards(S: np.ndarray, T: np.ndarray, w: np.ndarray):
    """Build per-core input maps (layout marshalling + tiny projections)."""
    bf16 = ml_dtypes.bfloat16
    ws, wt, wm = w[:D], w[D : 2 * D], w[2 * D :]
    sp = S @ ws  # [B, 512]
    tp = T @ wt  # [B, 512]
    # row blocks: i = 4p + ic
    A = S.reshape(B, 128, 4, D)
    Bt = T.reshape(B, 128, 4, D)
    # transposed cols: c = ic*128 + p  ->  i = 4p + ic ; d rows 3p+k for d<384
    StP = A.transpose(0, 3, 2, 1).reshape(B, D, 512)
    TtP = Bt.transpose(0, 3, 2, 1).reshape(B, D, 512) * wm[None, :, None]
    stt = StP[:, 0:384].reshape(B, 128, 3, 512)
    ttt = TtP[:, 0:384].reshape(B, 128, 3, 512)
    trow = np.empty((B, 128, 4, 401), np.float32)
    trow[:, :, :, 0:400] = Bt
    trow[:, :, :, 400] = 1.0
    mega = np.empty((B, 128, MEGA_W), np.float32)
    for k in range(3):
        mega[:, :, 1024 * k : 1024 * k + 512] = stt[:, :, k]
        mega[:, :, 1024 * k + 512 : 1024 * (k + 1)] = ttt[:, :, k]
    mega[:, :, OTR:MEGA_W] = trow.reshape(B, 128, 1604)
    mega = mega.astype(bf16)

    # aff rows; projections in c-order: c = ic*128+p <-> i = 4p+ic
    sp_c = sp.reshape(B, 128, 4).transpose(0, 2, 1).reshape(B, 512)
    tp_c = tp.reshape(B, 128, 4).transpose(0, 2, 1).reshape(B, 512)
    aff = np.empty((B, 18, 1024), np.float32)
    aff[:, 0:16, 0:512] = StP[:, 384:400]
    aff[:, 0:16, 512:1024] = TtP[:, 384:400]
    aff[:, 16, 0:512] = 1.0
    aff[:, 16, 512:1024] = tp_c
    aff[:, 17, 0:512] = sp_c
    aff[:, 17, 512:1024] = 1.0
    aff = aff.astype(bf16)

    in_maps = []
    for c in range(N_CORES):
        sl = slice(c * BL, (c + 1) * BL)
        in_maps.append({"mega": mega[sl], "aff": aff[sl]})
    return in_maps


def kernel(source_embedding, target_embedding, w_sim, **run_kwargs):
    S = np.asarray(source_embedding, dtype=np.float32)
    T = np.asarray(target_embedding, dtype=np.float32)
    w = np.asarray(w_sim, dtype=np.float32)
    assert S.shape == (B, LS, D) and T.shape == (B, LT, D) and w.shape == (3 * D,)

    nc = _get_program()
    in_maps = _host_shards(S, T, w)
    res = run_bass_kernel_spmd(nc, in_maps, core_ids=list(range(N_CORES)), **run_kwargs)

    out = np.empty((B, LS, 1600), np.float32)
    out[:, :, 0:400] = S
    for c in range(N_CORES):
        sl = slice(c * BL, (c + 1) * BL)
        po = (
            np.asarray(res.results[c]["po"])
            .astype(np.float32)
            .reshape(BL, 512, 401)
        )  # rows i = 4p+ic
        st = po[:, :, 0:400] / po[:, :, 400:401]
        u_c = np.asarray(res.results[c]["m"]).astype(np.float32).max(axis=1)
        u = u_c.reshape(BL, 4, 128).transpose(0, 2, 1).reshape(BL, 512)
        attn = u / u.sum(axis=1, keepdims=True)  # [BL, 512]
        ts = np.einsum("bi,bid->bd", attn, S[sl])  # [BL, 400]
        out[sl, :, 400:800] = st
        out[sl, :, 800:1200] = S[sl] * st
        out[sl, :, 1200:1600] = S[sl] * ts[:, None, :]
    if run_kwargs:
        kernel.last_results = res  # expose profile info to test harness
    return out
